# revision 10
# baseline (speedup 1.0000x reference)
"""ChebConv GNN (3 layers, K=5) on 8 Trainium2 NeuronCores.

Strategy (hardcoded for N=100000, E=1600000, F=128/128/32/40, K=5):
  - Clenshaw evaluation of sum_k T_k(L) h W_k  -> exactly K-1=4 sparse
    propagations per layer, each on an F_out-wide operand.
  - 1D node sharding: 128-node windows assigned to (core, pass, wpos)
    slots, balanced by edge count.  Every core runs the IDENTICAL program
    (SPMD); all per-core structure lives in input tables.
  - Propagation (segment-sum over dst-sorted edges) via PE matmuls:
    psum[window] += S_tile^T @ G_tile, where G_tile = dma_gather of 128
    source rows and S_tile[e, d] = (iota[d]==dstl[e]) * w[e] built by one
    fused DVE tensor_scalar op.
  - C_k = h @ W_k accumulated into the same PSUM tile (start=True).
  - AllGather of the propagated operand between Clenshaw steps.
  - For_i hardware loop over passes with a one-pass-shifted gather
    pipeline (gather pass i+1/i+2 while computing pass i/i+1).
"""

import sys
import os

sys.path.insert(0, "/opt/trn_rl_repo")

import numpy as np

# ---------------- problem constants (hardcoded; kernel.py must be
# self-contained and may not read spec.json/reference.py) ----------------
N = 100_000
E = 1_600_000
FIN = 128
HID = 128
F2 = 32
OUT = 40
K = 5

NCORES = 8
P = 128
GW = (N + P - 1) // P          # 782 global 128-node windows
WPP = 5                        # windows per pass
NPASS = 20                     # passes per propagation
WPC = WPP * NPASS              # 100 window slots per core
NR = WPC * P                   # 12800 rows per core shard
TROWS = NCORES * NR            # 102400 rows in gathered table
NCHUNK = 4
CH = TROWS // NCHUNK           # 25600 (< 32768, int16-addressable)

# per-layer config: (Fout, F_pad for gather elem, gather/matmul dtype)
F_PAD = {1: 128, 2: 64, 3: 64}
LAYER_FOUT = {1: HID, 2: F2, 3: OUT}
LAYER_FIN = {1: FIN, 2: HID, 3: F2}
# propagation dtype per layer: "f32" or "bf16"
PROP_DT = {1: os.environ.get("CHEB_L1_DT", "f32"), 2: "f32", 3: "f32"}
# debug: truncate after this many Clenshaw steps (-1 = full)
TRUNC = int(os.environ.get("CHEB_TRUNC", "-1"))


# =====================================================================
# Host-side graph preprocessing
# =====================================================================
def _prep(edge_index: np.ndarray):
    src = edge_index[0].astype(np.int64)
    dst = edge_index[1].astype(np.int64)

    deg = np.bincount(src, minlength=N).astype(np.float32)
    dis = np.where(deg > 0, 1.0 / np.sqrt(np.maximum(deg, 1.0)), 0.0).astype(
        np.float32
    )
    w1 = (-dis[src] * dis[dst]).astype(np.float32)

    # ----- window -> (core, pass, wpos) assignment, balanced by size -----
    gdst = dst // P
    wcnt = np.bincount(gdst, minlength=GW)
    order = np.argsort(-wcnt, kind="stable")          # ranks: big first
    # rank r -> wpos group of 160 (NCORES*NPASS) windows; within group,
    # spread across cores/passes.
    SLOT_G = NCORES * NPASS
    core_of_w = np.full(GW, -1, np.int64)
    pass_of_w = np.full(GW, -1, np.int64)
    wpos_of_w = np.full(GW, -1, np.int64)
    for r, g in enumerate(order):
        wpos = r // SLOT_G
        q = r % SLOT_G
        core_of_w[g] = q % NCORES
        pass_of_w[g] = q // NCORES
        wpos_of_w[g] = wpos
    pos_of_w = pass_of_w * WPP + wpos_of_w            # position in [0, WPC)

    # permutation: node -> row in the AllGathered table
    nodes = np.arange(N, dtype=np.int64)
    gs = nodes // P
    perm = core_of_w[gs] * NR + pos_of_w[gs] * P + (nodes - gs * P)

    # ----- per-edge keys -----
    e_core = core_of_w[gdst]
    e_pass = pass_of_w[gdst]
    e_wpos = wpos_of_w[gdst]
    e_chunk = perm[src] // CH
    e_dstl = (dst - gdst * P).astype(np.float32)
    e_idx = (perm[src] - e_chunk * CH).astype(np.int16)

    # cell = (core, pass, chunk, wpos) ; cells ordered (pass, chunk, wpos)
    # within each core to match the device's gather/tile order.
    cell = ((e_core * NPASS + e_pass) * NCHUNK + e_chunk) * WPP + e_wpos
    ncells = NCORES * NPASS * NCHUNK * WPP
    cnt = np.bincount(cell, minlength=ncells).reshape(NCORES, NPASS, NCHUNK, WPP)

    # budget B[wpos][chunk] = max tiles over (core, pass)
    tiles = (cnt + P - 1) // P                        # ceil
    B = tiles.max(axis=(0, 1)).T.copy()               # [WPP, NCHUNK] -> index [wpos][c]
    T_c = B.sum(axis=0)                               # tiles per (pass, chunk)
    T_P = int(T_c.sum())                              # tiles per pass

    # slot offsets: per core, per pass block of T_P*128 slots, cells in
    # (chunk, wpos) order inside the pass.
    cell_sizes = np.zeros((NCORES, NPASS, NCHUNK, WPP), np.int64)
    cell_sizes[:, :, :, :] = (B.T[None, None] * P)    # B.T = [c][wpos]
    flat_sizes = cell_sizes.reshape(-1)
    cell_off = np.zeros(ncells, np.int64)
    cell_off[1:] = np.cumsum(flat_sizes)[:-1]
    # offsets are relative to each core's slot array: subtract core base
    core_base = cell_off.reshape(NCORES, -1)[:, 0].copy()

    nslots_core = NPASS * T_P * P

    # position of each edge within its cell
    sort_idx = np.argsort(cell, kind="stable")
    cell_sorted = cell[sort_idx]
    starts = np.searchsorted(cell_sorted, np.arange(ncells))
    within = np.arange(E, dtype=np.int64) - starts[cell_sorted]
    out_pos_sorted = cell_off[cell_sorted] + within
    out_pos = np.empty(E, np.int64)
    out_pos[sort_idx] = out_pos_sorted

    # global slot arrays (all cores concatenated), padded defaults
    tot_slots = NCORES * nslots_core
    s_idx = np.zeros(tot_slots, np.int16)
    s_dstl = np.zeros(tot_slots, np.float32)
    s_w = np.zeros(tot_slots, np.float32)
    s_idx[out_pos] = e_idx
    s_dstl[out_pos] = e_dstl
    s_w[out_pos] = w1

    # ----- pack per-core device tables -----
    idx_tbls, meta1_tbls, meta2_tbls = [], [], []
    for c in range(NCORES):
        lo = c * nslots_core
        ci = s_idx[lo:lo + nslots_core].reshape(NPASS, T_P * P)
        cd = s_dstl[lo:lo + nslots_core].reshape(NPASS, T_P, P)
        cw = s_w[lo:lo + nslots_core].reshape(NPASS, T_P, P)
        # idx wrap: idx j -> [j % 16, j // 16]; plus one dummy pass block
        blocks = []
        for i in range(NPASS):
            blocks.append(ci[i].reshape(T_P * 8, 16).T)     # [16, T_P*8]
        blocks.append(np.zeros((16, T_P * 8), np.int16))    # dummy pass
        idx2 = np.concatenate(blocks, axis=1)               # [16, (NPASS+1)*T_P*8]
        idx_tbls.append(np.tile(idx2, (8, 1)))              # [128, ...]

        def pack_meta(warr):
            m = np.zeros((NPASS, P, 2 * T_P), np.float32)
            for i in range(NPASS):
                m[i, :, 0::2] = cd[i].T                     # dstl cols
                m[i, :, 1::2] = warr[i].T                   # w cols
            return m.transpose(1, 0, 2).reshape(P, NPASS * 2 * T_P)

        meta1_tbls.append(pack_meta(cw))
        meta2_tbls.append(pack_meta(2.0 * cw))

    info = dict(B=B, T_c=T_c, T_P=T_P, perm=perm,
                core_of_w=core_of_w, pos_of_w=pos_of_w,
                idx=idx_tbls, meta1=meta1_tbls, meta2=meta2_tbls)
    return info


def _permute_x(x: np.ndarray, info):
    """x [N, FIN] -> per-core [NR, FIN] shards in slot order."""
    xs = [np.zeros((NR, FIN), np.float32) for _ in range(NCORES)]
    core_of_w, pos_of_w = info["core_of_w"], info["pos_of_w"]
    for g in range(GW):
        c, p = core_of_w[g], pos_of_w[g]
        lo, hi = g * P, min((g + 1) * P, N)
        xs[c][p * P:p * P + (hi - lo)] = x[lo:hi]
    return xs


def _assemble_out(shards, info):
    out = np.zeros((N, OUT), np.float32)
    core_of_w, pos_of_w = info["core_of_w"], info["pos_of_w"]
    for g in range(GW):
        c, p = core_of_w[g], pos_of_w[g]
        lo, hi = g * P, min((g + 1) * P, N)
        out[lo:hi] = shards[c][p * P:p * P + (hi - lo), :OUT]
    return out


# =====================================================================
# Numpy emulation of the device algorithm (validates tables + algorithm)
# =====================================================================
def _emu_prop(info, tbl, w_sel, fpad):
    """tbl: [TROWS, fpad] gathered table; returns per-core [NR, fpad]."""
    T_P = info["T_P"]
    outs = []
    for c in range(NCORES):
        idx = info["idx"][c][:16]                      # [16, (NPASS+1)*T_P*8]
        meta = info[w_sel][c]                          # [128, NPASS*2*T_P]
        out = np.zeros((NR, fpad), np.float32)
        for i in range(NPASS):
            ib = idx[:, i * T_P * 8:(i + 1) * T_P * 8]     # [16, T_P*8]
            idxs = ib.T.reshape(-1).astype(np.int64)       # T_P*128 wrapped order
            mb = meta[:, i * 2 * T_P:(i + 1) * 2 * T_P]
            # chunk base offsets
            off = 0
            for ci in range(NCHUNK):
                Tc = int(info["B"][:, ci].sum())
                for t in range(Tc):
                    tg = off + t
                    eidx = idxs[tg * P:(tg + 1) * P]
                    # which wpos does tile t of chunk ci belong to?
                    # tiles ordered by wpos with B[wpos][ci] tiles each
                    acc, wp = 0, 0
                    while t >= acc + info["B"][wp, ci]:
                        acc += info["B"][wp, ci]
                        wp += 1
                    dstl = mb[:, 2 * tg].astype(np.int64)
                    wv = mb[:, 2 * tg + 1]
                    g = tbl[ci * CH + eidx]               # [128, fpad]
                    z = g * wv[:, None]
                    rows = (i * WPP + wp) * P + dstl
                    np.add.at(out, rows, z)
                off += Tc
        outs.append(out)
    return outs


def _emulate(x, info, W1, b1, W2, b2, W3, b3):
    """Full numpy emulation with device data layout. Returns [N, OUT]."""
    mean = x.mean(axis=0)
    std = x.std(axis=0, ddof=1)
    xs = _permute_x(x, info)
    hs = [(s - mean) / std for s in xs]
    # zero out pad rows again (they were 0 - mean...)
    # (pad rows only pollute pad outputs; harmless, keep consistent w/ device)

    def layer(hs, W, bias, fpad, relu):
        Kk = W.shape[0]
        C = [[h @ W[k] for h in hs] for k in range(Kk)]
        fout = W.shape[2]

        def pad(mats):
            return [np.concatenate([m, np.zeros((NR, fpad - fout), np.float32)], 1)
                    for m in mats]

        def ag(mats):
            return np.concatenate(mats, axis=0)

        b_kp1 = None  # b_{k+1} per-core (padded)
        b_kp2 = None
        for s in range(Kk):          # s=0 -> k=K-1 ... s=K-1 -> k=0 (final)
            k = Kk - 1 - s
            if s == 0:
                b_k = pad(C[k])
            else:
                w_sel = "meta1" if s == Kk - 1 else "meta2"
                prop = _emu_prop(info, ag(b_kp1), w_sel, fpad)
                b_k = []
                for c in range(NCORES):
                    v = prop[c]
                    v[:, :fout] += C[k][c]
                    if b_kp2 is not None:
                        v = v - b_kp2[c]
                    b_k.append(v)
            b_kp2, b_kp1 = b_kp1, b_k
        outs = []
        for c in range(NCORES):
            v = b_kp1[c][:, :fout] + bias[None, :]
            if relu:
                v = np.maximum(v, 0.0)
            outs.append(v)
        return outs

    hs = layer(hs, W1, b1, F_PAD[1], True)
    hs = layer(hs, W2, b2, F_PAD[2], True)
    hs = layer(hs, W3, b3, F_PAD[3], False)
    return _assemble_out(hs, info)


# =====================================================================
# Bass kernel
# =====================================================================
def _build_nc(info):
    import concourse.bass as bass
    import concourse.mybir as mybir
    import concourse.tile as tile
    from concourse import bacc
    from concourse.bass import ds

    f32 = mybir.dt.float32
    bf16 = mybir.dt.bfloat16
    DT = {"f32": f32, "bf16": bf16}

    T_P = info["T_P"]
    B = info["B"]                        # [WPP, NCHUNK]
    T_c = [int(x) for x in info["T_c"]]  # tiles per chunk per pass

    nc = bacc.Bacc(None, target_bir_lowering=False)

    # ---- I/O ----
    xs_d = nc.dram_tensor("xs", [NR, FIN], f32, kind="ExternalInput")
    idx_d = nc.dram_tensor("idx", [P, (NPASS + 1) * T_P * 8], mybir.dt.int16,
                           kind="ExternalInput")
    meta_d = {}
    for nm in ("meta1", "meta2"):
        meta_d[nm] = nc.dram_tensor(nm, [P, NPASS * 2 * T_P], f32,
                                    kind="ExternalInput")
    # consts: iota | identity | bias1(128) | bias2(32) | bias3(40) | ones(1)
    CW = 128 + 128 + 128 + 32 + 40 + 1
    consts_d = nc.dram_tensor("consts", [P, CW], f32, kind="ExternalInput")
    w_d = {
        1: nc.dram_tensor("w1", [FIN, K * HID], f32, kind="ExternalInput"),
        2: nc.dram_tensor("w2", [HID, K * F2], f32, kind="ExternalInput"),
        3: nc.dram_tensor("w3", [F2, K * OUT], f32, kind="ExternalInput"),
    }
    out_d = nc.dram_tensor("out_shard", [NR, OUT], f32, kind="ExternalOutput")

    # ---- internal DRAM ----
    # shard/table buffers per row byte-width (512B for f32 L1, 256B for
    # f32-padded L2/L3 or bf16 L1)
    tsh = {w: [nc.dram_tensor(f"tsh{w}_{j}", [NR, w // 4], f32)
               for j in range(2)] for w in (512, 256)}
    tbuf = {w: [nc.dram_tensor(f"tbuf{w}_{j}", [TROWS, w // 4], f32,
                               addr_space="Shared")
                for j in range(2)] for w in (512, 256)}
    st_in = nc.dram_tensor("st_in", [P, 2], f32)
    st_out = nc.dram_tensor("st_out", [P, 2], f32, addr_space="Shared")

    RG = [[0, 1, 2, 3, 4, 5, 6, 7]]

    with tile.TileContext(nc) as tc:
        with tc.tile_pool(name="per", bufs=1) as per, \
             tc.tile_pool(name="big", bufs=1) as bigp, \
             tc.tile_pool(name="str", bufs=2) as strm, \
             tc.tile_pool(name="Sp", bufs=4) as Sp, \
             tc.tile_pool(name="gb", bufs=1) as gbp, \
             tc.tile_pool(name="ps", bufs=5, space="PSUM") as psp, \
             tc.tile_pool(name="pst", bufs=1, space="PSUM") as pst:

            # ---------- persistent constants ----------
            consts = per.tile([P, CW], f32, tag="consts")
            nc.sync.dma_start(consts[:], consts_d[:])
            iota_f32 = consts[:, 0:128]
            ident = consts[:, 128:256]
            biases = {1: consts[:, 256:384], 2: consts[:, 384:416],
                      3: consts[:, 416:456]}
            ones_col = consts[:, 456:457]

            wmat = {}
            for l in (1, 2, 3):
                fin = LAYER_FIN[l]
                wm = per.tile([P, K * LAYER_FOUT[l]], f32, tag=f"wm{l}")
                nc.sync.dma_start(wm[:fin, :], w_d[l][:])
                wmat[l] = wm

            iota_bf = per.tile([P, 128], bf16, tag="iotabf")
            nc.vector.tensor_copy(iota_bf[:], iota_f32)

            # hT master buffer [128, WPC*128] f32 (bitcast views for bf16)
            hT = bigp.tile([P, WPC * P], f32, tag="hT")

            # ---------- phase A: stats + transpose of x ----------
            ps_stat = pst.tile([P, 2], f32, space="PSUM", tag="stat")
            for p in range(WPC):
                xt = strm.tile([P, FIN], f32, tag="xt")
                nc.sync.dma_start(xt[:], xs_d[p * P:(p + 1) * P, :])
                sq = strm.tile([P, FIN], f32, tag="sq")
                nc.vector.tensor_tensor(out=sq[:], in0=xt[:], in1=xt[:],
                                        op=mybir.AluOpType.mult)
                nc.tensor.matmul(ps_stat[:, 0:1], lhsT=xt[:], rhs=ones_col,
                                 start=(p == 0), stop=False)
                nc.tensor.matmul(ps_stat[:, 1:2], lhsT=sq[:], rhs=ones_col,
                                 start=(p == 0), stop=(p == WPC - 1))
                ps_t = pst.tile([P, P], f32, space="PSUM", tag="tp", bufs=2)
                nc.tensor.transpose(out=ps_t[:], in_=xt[:], identity=ident)
                nc.vector.tensor_copy(hT[:, p * P:(p + 1) * P], ps_t[:])

            stat_sb = per.tile([P, 2], f32, tag="statsb")
            nc.vector.tensor_copy(stat_sb[:], ps_stat[:])
            nc.sync.dma_start(st_in[:], stat_sb[:])
            nc.gpsimd.collective_compute("AllReduce", mybir.AluOpType.add,
                                         replica_groups=RG,
                                         ins=[st_in[:]], outs=[st_out[:]])
            stat2 = per.tile([P, 2], f32, tag="stat2")
            nc.sync.dma_start(stat2[:], st_out[:])
            # mean = s/N ; var = sq/(N-1) - s^2/(N(N-1)) ; rstd = rsqrt(var)
            mean = per.tile([P, 1], f32, tag="mean")
            nc.vector.tensor_scalar(out=mean[:], in0=stat2[:, 0:1],
                                    scalar1=1.0 / N, scalar2=None,
                                    op0=mybir.AluOpType.mult)
            va = per.tile([P, 1], f32, tag="va")
            nc.vector.tensor_scalar(out=va[:], in0=stat2[:, 1:2],
                                    scalar1=1.0 / (N - 1), scalar2=None,
                                    op0=mybir.AluOpType.mult)
            vb = per.tile([P, 1], f32, tag="vb")
            nc.vector.tensor_tensor(out=vb[:], in0=stat2[:, 0:1],
                                    in1=stat2[:, 0:1], op=mybir.AluOpType.mult)
            nc.vector.tensor_scalar(out=vb[:], in0=vb[:],
                                    scalar1=1.0 / (float(N) * (N - 1)),
                                    scalar2=None, op0=mybir.AluOpType.mult)
            nc.vector.tensor_tensor(out=va[:], in0=va[:], in1=vb[:],
                                    op=mybir.AluOpType.subtract)
            rstd = per.tile([P, 1], f32, tag="rstd")
            nc.scalar.activation(rstd[:], va[:],
                                 mybir.ActivationFunctionType.Sqrt)
            nc.vector.reciprocal(rstd[:], rstd[:])
            # standardize hT in place: (x - mean) * rstd, per partition
            nc.vector.tensor_scalar(out=hT[:], in0=hT[:], scalar1=mean[:],
                                    scalar2=rstd[:],
                                    op0=mybir.AluOpType.subtract,
                                    op1=mybir.AluOpType.mult)

            # ---------- per-layer Clenshaw ----------
            def run_layer(l, relu):
                fin = LAYER_FIN[l]
                fout = LAYER_FOUT[l]
                fpad = F_PAD[l]
                dt = DT[PROP_DT[l]]
                is_bf = (dt == bf16)
                esz = fpad  # elem_size in elements
                iota_t = iota_bf if is_bf else iota_f32
                dsz = 2 if is_bf else 4
                WB = fpad * dsz  # row bytes

                # layer-local views of shard/table DRAM at fpad width
                def tsh_v(j):
                    t = tsh[WB][j][:]
                    if is_bf:
                        t = t.bitcast(bf16)
                    return t  # [NR, fpad]

                def tbuf_v(j):
                    t = tbuf[WB][j][:]
                    if is_bf:
                        t = t.bitcast(bf16)
                    return t

                # ---- step s: one Clenshaw step ----
                # s=0: b = C_k      (no prop, no For_i)
                # s>=1: b = prop(2L or L) + C_k [- b_prev]
                def step(s):
                    k = K - 1 - s
                    final = (s == K - 1)
                    wsel = "meta1" if final else "meta2"
                    cur = s % 2          # t-shard parity this step writes
                    src = (s - 1) % 2    # parity propagated (written by s-1)
                    prevp = s % 2        # parity of b_{k+2} = step s-2 output

                    if s == 0:
                        for i in range(NPASS):
                            _pass_body_s0(i)
                        _ag(cur)
                        return

                    # prologue gather for pass 0
                    _gathers(0, src, 0)
                    with tc.For_i(0, NPASS, 2) as i:
                        _gathers(i + 1, src, 1)
                        _compute(i, 0, s, k, final, wsel, cur, prevp)
                        _gathers(i + 2, src, 0)
                        _compute(i + 1, 1, s, k, final, wsel, cur, prevp)
                    if not final:
                        _ag(cur)

                def _ag(j):
                    nc.gpsimd.collective_compute(
                        "AllGather", mybir.AluOpType.bypass, replica_groups=RG,
                        ins=[tsh[WB][j][:]], outs=[tbuf[WB][j][:]])

                def _gathers(i, src, par):
                    # i may be a For_i expression; par = ping/pong
                    idx_t = gb_idx[par]
                    nc.sync.dma_start(
                        idx_t[:], idx_d[:, ds(i * (T_P * 8), T_P * 8)])
                    off = 0
                    for c in range(NCHUNK):
                        tc_ = T_c[c]
                        if tc_ == 0:
                            continue
                        nc.gpsimd.dma_gather(
                            out_ap=gbuf[par][:, off:off + tc_, :],
                            in_ap=tbuf_v(src)[c * CH:(c + 1) * CH, :esz],
                            idxs_ap=idx_t[:, off * 8:(off + tc_) * 8],
                            num_idxs=tc_ * P,
                            num_idxs_reg=tc_ * P,
                            elem_size=esz,
                            single_packet=False,
                        )
                        off += tc_

                def _pass_body_s0(i):
                    # b = C_k only (k = K-1); static python pass index
                    hs = strm.tile([P, WPP * P], f32, tag="hstg")
                    nc.sync.dma_start(hs[:fin, :],
                                      hT[:fin, i * WPP * P:(i + 1) * WPP * P])
                    hsd = hs
                    if is_bf:
                        hsd = strm.tile([P, WPP * P], bf16, tag="hstgb")
                        nc.vector.tensor_copy(hsd[:fin, :], hs[:fin, :])
                    wmd = wm_l
                    stage = strm.tile([P, WPP * fpad], f32, tag="stg")
                    for wp in range(WPP):
                        pw = psp.tile([P, fpad], f32, space="PSUM", tag="pw")
                        nc.tensor.matmul(
                            pw[:, :fout],
                            lhsT=hsd[:fin, wp * P:(wp + 1) * P],
                            rhs=wmd[:fin, (K - 1) * fout:K * fout],
                            start=True, stop=True)
                        nc.vector.tensor_copy(stage[:, wp * fpad:(wp + 1) * fpad],
                                              pw[:])
                    stg_o = stage
                    if is_bf:
                        stg_o = strm.tile([P, WPP * fpad], bf16, tag="stgb")
                        nc.vector.tensor_copy(stg_o[:], stage[:])
                    nc.sync.dma_start(
                        tsh_v(0).rearrange("(w p) f -> p w f", p=P)[
                            :, i * WPP:(i + 1) * WPP, :esz],
                        stg_o[:].rearrange("p (w f) -> p w f", f=fpad))

                def _compute(i, par, s, k, final, wsel, cur, prevp):
                    # pass i compute: C_k matmuls + S-matmuls + flush
                    hs = strm.tile([P, WPP * P], f32, tag="hstg")
                    nc.sync.dma_start(hs[:fin, :],
                                      hT[:fin, ds(i * (WPP * P), WPP * P)])
                    hsd = hs
                    if is_bf:
                        hsd = strm.tile([P, WPP * P], bf16, tag="hstgb")
                        nc.vector.tensor_copy(hsd[:fin, :], hs[:fin, :])
                    meta_t = strm.tile([P, 2 * T_P], f32, tag="meta")
                    nc.sync.dma_start(
                        meta_t[:], meta_d[wsel][:, ds(i * (2 * T_P), 2 * T_P)])
                    meta_v = meta_t
                    if is_bf:
                        meta_v = strm.tile([P, 2 * T_P], bf16, tag="metab")
                        nc.vector.tensor_copy(meta_v[:], meta_t[:])
                    if s >= 2:
                        prev = strm.tile([P, WPP * fpad], f32, tag="prev")
                        pv = tsh_v(prevp).rearrange("(w p) f -> p w f", p=P)[
                            :, ds(i * WPP, WPP), :esz]
                        if is_bf:
                            prevb = strm.tile([P, WPP * fpad], bf16, tag="prevb")
                            nc.sync.dma_start(
                                prevb[:].rearrange("p (w f) -> p w f", f=fpad), pv)
                            nc.vector.tensor_copy(prev[:], prevb[:])
                        else:
                            nc.sync.dma_start(
                                prev[:].rearrange("p (w f) -> p w f", f=fpad), pv)

                    last_c = [max([c for c in range(NCHUNK) if B[wp, c] > 0],
                                  default=-1) for wp in range(WPP)]
                    pws = []
                    for wp in range(WPP):
                        pw = psp.tile([P, fpad], f32, space="PSUM", tag="pw")
                        pws.append(pw)
                        nc.tensor.matmul(
                            pw[:, :fout],
                            lhsT=hsd[:fin, wp * P:(wp + 1) * P],
                            rhs=wm_l[:fin, k * fout:(k + 1) * fout],
                            start=True, stop=(last_c[wp] < 0))
                    # S-matmuls: tiles ordered (chunk, wpos, j)
                    off = 0
                    for c in range(NCHUNK):
                        for wp in range(WPP):
                            nt = int(B[wp, c])
                            for j in range(nt):
                                tg = off
                                off += 1
                                S = Sp.tile([P, P], dt, tag="S")
                                nc.vector.tensor_scalar(
                                    out=S[:], in0=iota_t,
                                    scalar1=meta_v[:, 2 * tg:2 * tg + 1],
                                    scalar2=meta_v[:, 2 * tg + 1:2 * tg + 2],
                                    op0=mybir.AluOpType.is_equal,
                                    op1=mybir.AluOpType.mult)
                                last = (c == last_c[wp]) and (j == nt - 1)
                                nc.tensor.matmul(
                                    pws[wp][:], lhsT=S[:],
                                    rhs=gbuf[par][:, tg, :],
                                    start=False, stop=last)
                    # flush
                    stage = strm.tile([P, WPP * fpad], f32, tag="stg")
                    for wp in range(WPP):
                        dst_sl = stage[:, wp * fpad:(wp + 1) * fpad]
                        if s >= 2:
                            nc.vector.tensor_tensor(
                                out=dst_sl, in0=pws[wp][:],
                                in1=prev[:, wp * fpad:(wp + 1) * fpad],
                                op=mybir.AluOpType.subtract)
                        else:
                            nc.vector.tensor_copy(dst_sl, pws[wp][:])
                        if final:
                            nc.vector.tensor_tensor(
                                out=dst_sl[:, :fout], in0=dst_sl[:, :fout],
                                in1=biases[l][:, :fout],
                                op=mybir.AluOpType.add)
                            if relu:
                                nc.vector.tensor_scalar(
                                    out=dst_sl[:, :fout], in0=dst_sl[:, :fout],
                                    scalar1=0.0, scalar2=None,
                                    op0=mybir.AluOpType.max)
                    if not final:
                        stg_o = stage
                        if is_bf:
                            stg_o = strm.tile([P, WPP * fpad], bf16, tag="stgb")
                            nc.vector.tensor_copy(stg_o[:], stage[:])
                        nc.sync.dma_start(
                            tsh_v(cur).rearrange("(w p) f -> p w f", p=P)[
                                :, ds(i * WPP, WPP), :esz],
                            stg_o[:].rearrange("p (w f) -> p w f", f=fpad))
                    else:
                        if l == 3:
                            nc.sync.dma_start(
                                out_d[:].rearrange("(w p) f -> p w f", p=P)[
                                    :, ds(i * WPP, WPP), :],
                                stage[:].rearrange("p (w f) -> p w f",
                                                   f=fpad)[:, :, :OUT])
                        else:
                            # build next layer hT in place: transpose each
                            # window's [128, fout] block
                            for wp in range(WPP):
                                ps_t = pst.tile([P, P], f32, space="PSUM",
                                                tag="tp", bufs=2)
                                nc.tensor.transpose(
                                    out=ps_t[:fout, :],
                                    in_=stage[:, wp * fpad:wp * fpad + fout],
                                    identity=ident)
                                htn = strm.tile([P, P], f32, tag="htn")
                                nc.vector.tensor_copy(htn[:fout, :], ps_t[:fout, :])
                                nc.sync.dma_start(
                                    hT[:fout, ds((i * WPP + wp) * P, P)],
                                    htn[:fout, :])

                # gather buffers (ping/pong) + idx tiles, allocated per layer
                gbuf = []
                gb_idx = []
                for j in range(2):
                    gtile = gbp.tile([P, T_P, esz], dt, tag=f"gb{j}")
                    gbuf.append(gtile)
                    gitile = gbp.tile([P, T_P * 8], mybir.dt.int16,
                                      tag=f"gi{j}")
                    gb_idx.append(gitile)
                wm_l = wmat[l]
                if is_bf:
                    wmb = per.tile([P, K * fout], bf16, tag=f"wmb{l}")
                    nc.vector.tensor_copy(wmb[:fin, :], wm_l[:fin, :])
                    wm_l = wmb

                for s in range(K):
                    if TRUNC >= 0 and _steps_done[0] >= TRUNC:
                        return
                    step(s)
                    _steps_done[0] += 1

            _steps_done = [0]
            run_layer(1, relu=True)
            run_layer(2, relu=True)
            run_layer(3, relu=False)

    nc.compile()
    return nc


# =====================================================================
# Entry point
# =====================================================================
def kernel(x, edge_index, W1, b1, W2, b2, W3, b3):
    from concourse.bass_utils import run_bass_kernel_spmd

    x = np.asarray(x, np.float32)
    info = _prep(np.asarray(edge_index))
    xs = _permute_x(x, info)

    iota = np.tile(np.arange(P, dtype=np.float32)[None, :], (P, 1))
    ident = np.eye(P, dtype=np.float32)
    CW = 128 + 128 + 128 + 32 + 40 + 1
    consts = np.zeros((P, CW), np.float32)
    consts[:, 0:128] = iota
    consts[:, 128:256] = ident
    consts[:, 256:256 + HID] = np.tile(np.asarray(b1, np.float32)[None, :], (P, 1))
    consts[:, 384:384 + F2] = np.tile(np.asarray(b2, np.float32)[None, :], (P, 1))
    consts[:, 416:416 + OUT] = np.tile(np.asarray(b3, np.float32)[None, :], (P, 1))
    consts[:, 456] = 1.0

    w1m = np.ascontiguousarray(
        np.asarray(W1, np.float32).transpose(1, 0, 2).reshape(FIN, K * HID))
    w2m = np.ascontiguousarray(
        np.asarray(W2, np.float32).transpose(1, 0, 2).reshape(HID, K * F2))
    w3m = np.ascontiguousarray(
        np.asarray(W3, np.float32).transpose(1, 0, 2).reshape(F2, K * OUT))

    nc = _build_nc(info)
    in_maps = []
    for c in range(NCORES):
        in_maps.append({
            "xs": xs[c], "idx": info["idx"][c],
            "meta1": info["meta1"][c], "meta2": info["meta2"][c],
            "consts": consts, "w1": w1m, "w2": w2m, "w3": w3m,
        })
    res = run_bass_kernel_spmd(nc, in_maps, list(range(NCORES)))
    shards = [res.results[c]["out_shard"] for c in range(NCORES)]
    return _assemble_out(shards, info)
